# revision 21
# baseline (speedup 1.0000x reference)
"""Trainium2 Bass kernel for nn_Cube_Norm (segment min/max normalize).

Reference semantics (per graph g of 256 nodes, per dim d):
    tmax = max_n x[g,n,d]; tmin = min_n x[g,n,d]
    mid = (tmax+tmin)/2; ldv = max((tmax-tmin)/2, 1e-12)
    out[g,n,d] = (x[g,n,d] - mid) / ldv

Sharding: 1024 graphs -> 8 cores x 128 graphs (row-sharded at graph
boundaries). Per core, 4 rounds of 32 graphs; each graph occupies 4 SBUF
partitions (64 nodes each): every round is a [128, 19200] fp32 tile with
contiguous DMA in/out (exactly 2x HBM traffic), double-buffered.

Engine split (HW-probed on this silicon):
  - A DVE op with two SBUF operands (or an sb->sb copy) crawls 4-10x
    while GpSimd is busy: they arbitrate a shared SBUF port pair and the
    loser blocks for the whole instruction. A DVE op with one SBUF
    stream + PSUM for the rest runs at FULL speed alongside GpSimd.
  - So: DVE folds are PSUM-accumulator chains TT(chunk_sb, acc_pm ->
    acc_pm); the cross-partition stat tree + math run on PSUM scratch
    (with small ACT sb<->psum copies; ACT has its own ports); the
    DVE normalize slice reads stats from PSUM. GpSimd runs the bulk of
    the normalize from SBUF concurrently; in round 0 (no normalize yet)
    it folds a share of the chunks instead.
  - All stat math is TT/reciprocal with broadcast const tiles - never
    tensor_scalar/copy on DVE (2-port modes would grab the shared pair).
  - Loads ride the sync HWDGE ring; stores and stat DMAs ride the
    scalar ring, so stores never head-of-line-block loads.
"""

import numpy as np

NUM_GRAPHS = 1024
NPG = 256            # nodes per graph
D = 300              # embed dim
N_CORES = 8
GPC = NUM_GRAPHS // N_CORES   # 128 graphs per core
ROWS_PER_CORE = GPC * NPG     # 32768
P = 128              # SBUF partitions
Q = 4                # partitions per graph
NPP = NPG // Q       # 64 nodes per partition
GPR = P // Q         # 32 graphs per round
ROUNDS = GPC // GPR  # 4
FREE = NPP * D       # 19200 fp32 per partition per round
ROWS_PER_ROUND = GPR * NPG    # 8192
EPS = 1e-12

CH = 1200            # fold chunk (PSUM acc width; 300*2^k)
NCH = FREE // CH     # 16 chunks

# normalize node-split: DVE handles ND_* nodes of 64, GpSimd the rest
ND_MID = 17
ND_LAST = 43

_CACHE = {}


def _split_multi_waits(nc, mybir, max_waits=1):
    """walrus in this container rejects >N sync waits on one instruction;
    hoist extras into standalone NOPs on the same engine just before."""
    n = 0
    for f in nc.m.functions:
        for bb in f.blocks:
            new_insts = []
            for inst in bb.instructions:
                si = inst.sync_info
                if si is not None and si.on_wait and len(si.on_wait) > max_waits:
                    extra = list(si.on_wait[: len(si.on_wait) - max_waits])
                    keep = list(si.on_wait[len(si.on_wait) - max_waits:])
                    for j, w in enumerate(extra):
                        new_insts.append(
                            mybir.InstNoOp(
                                name=f"{inst.name}-sw{j}",
                                sync_info=mybir.SyncInfo(on_wait=[w], on_update=[]),
                                bass_nofuse=True,
                                engine=inst.engine,
                            )
                        )
                        n += 1
                    inst.sync_info = mybir.SyncInfo(
                        on_wait=keep, on_update=list(si.on_update)
                    )
                new_insts.append(inst)
            bb.instructions.clear()
            for i in new_insts:
                bb.add_instruction(i)
    return n


def _build():
    import concourse.bass as bass
    import concourse.tile as tile
    from concourse import mybir

    F32 = mybir.dt.float32
    OP = mybir.AluOpType

    nc = bass.Bass()
    x = nc.dram_tensor("x", [ROWS_PER_CORE, D], F32, kind="ExternalInput")
    y = nc.dram_tensor("y", [ROWS_PER_CORE, D], F32, kind="ExternalOutput")

    with tile.TileContext(nc) as tc:
        with tc.tile_pool(name="data", bufs=2) as data_pool, \
             tc.tile_pool(name="rep", bufs=2) as rep_pool, \
             tc.tile_pool(name="sml", bufs=1) as sml_pool, \
             tc.tile_pool(name="acc", bufs=2, space="PSUM") as acc_pool, \
             tc.tile_pool(name="prep", bufs=1, space="PSUM") as prep_pool:
            # broadcast consts for stat math ([GPR,1], read via rd0)
            cst = sml_pool.tile([GPR, 4], F32, tag="cst")
            nc.vector.memset(cst[:, 0:1], 0.5)
            nc.vector.memset(cst[:, 1:2], -0.5)
            nc.vector.memset(cst[:, 2:3], EPS)
            half_b = cst[:, 0:1].broadcast_to([GPR, D])
            neghalf_b = cst[:, 1:2].broadcast_to([GPR, D])
            eps_b = cst[:, 2:3].broadcast_to([GPR, D])

            # persistent PSUM stats [P, mid|rinv] (single buffer: its
            # reader (norm r-1) and writer (stats r) are both in-order
            # DVE ops, so WAR needs no extra buffering)
            pm_rep = prep_pool.tile([P, 2 * D], F32, tag="pmrep")

            live = {}  # r -> (t, rep_sb, parity) awaiting normalize+store
            for r in range(ROUNDS + 1):
                if r < ROUNDS:
                    rows = slice(r * ROWS_PER_ROUND, (r + 1) * ROWS_PER_ROUND)

                    # load in four quarters so folds start as data streams in
                    t = data_pool.tile([P, FREE], F32, tag="t")
                    xr = x[rows, :].rearrange("(p f) d -> p (f d)", p=P)
                    FQ = FREE // 4
                    for qd in range(4):
                        nc.sync.dma_start(
                            t[:, qd * FQ:(qd + 1) * FQ], xr[:, qd * FQ:(qd + 1) * FQ]
                        )

                    # per-partition partials: s cols [0:D]=max, [D:2D]=min.
                    # DVE chain keeps the accumulator in PSUM (in0 = sbuf
                    # chunk via the dedicated read port, in1/out = PSUM), so
                    # it never touches the DVE/GpSimd shared SBUF ports.
                    # ACT also fully blocks under GpSimd, so its two acc-init
                    # copies are issued here at the round head, inside the
                    # window where GpSimd still waits for this round's rep
                    # replication. Everything else PSUM->SBUF is DVE.
                    # (GpSimd can't help fold: Pool TT rejects max/min ops.)
                    s = sml_pool.tile([P, 2 * D], F32, tag="s")
                    accs = {}
                    for si in (0, 1):
                        accs[si] = acc_pool.tile(
                            [P, CH], F32, tag="acc", name=f"acc{r}_{si}"
                        )
                        nc.scalar.copy(accs[si][:], t[:, 0:CH])
                    for si, op in ((0, OP.max), (1, OP.min)):
                        acc = accs[si]
                        for c in range(1, NCH):
                            nc.vector.tensor_tensor(
                                acc[:], t[:, c * CH:(c + 1) * CH], acc[:], op=op
                            )
                        h = sml_pool.tile([P, CH // 2], F32, tag="h")
                        m = CH // 2
                        while m > D:
                            nc.vector.tensor_copy(h[:, 0:m], acc[:, m:2 * m])
                            nc.vector.tensor_tensor(
                                acc[:, 0:m], h[:, 0:m], acc[:, 0:m], op=op
                            )
                            m //= 2
                        nc.vector.tensor_copy(h[:, 0:D], acc[:, D:2 * D])
                        nc.vector.tensor_tensor(
                            s[:, si * D:(si + 1) * D], h[:, 0:D],
                            acc[:, 0:D], op=op,
                        )

                    # gather the 4 partials of each graph onto one partition.
                    # Scalar ring: the sync ring is reserved for loads so the
                    # next round's loads are never head-of-line-blocked; the
                    # scalar ring carries only early-completing work (acc
                    # inits, the DVE-slice store, gather, replicate).
                    tq = sml_pool.tile([GPR, Q, 2 * D], F32, tag="tq")
                    for q in range(Q):
                        nc.scalar.dma_start(tq[:, q, :], s[q::Q, :])

                if r >= 1:
                    # normalize round r-1: out = (x - mid) * rinv, in place.
                    # DVE slices read stats from PSUM (shared-pair-free) and
                    # store via the scalar ring (they complete early, so they
                    # can't block gather/replicate behind them). GpSimd slices
                    # read stats from SBUF and store via their own SWDGE ring
                    # right after their compute - keeping late-completing
                    # stores off both HWDGE rings entirely.
                    tp, rep_sb = live.pop(r - 1)
                    rowsp = slice((r - 1) * ROWS_PER_ROUND, r * ROWS_PER_ROUND)
                    tv3 = tp[:].rearrange("p (n d) -> p n d", n=NPP, d=D)
                    yr = y[rowsp, :].rearrange("(p f) d -> p (f d)", p=P)

                    nd = ND_LAST if r == ROUNDS else ND_MID
                    ng1 = (NPP - nd + 1) // 2
                    if r == ROUNDS:
                        # drain round: chunk the DVE slice so stores overlap
                        # the remaining compute instead of a serial tail
                        dsegs = [(a, min(a + 15, nd)) for a in range(0, nd, 15)]
                    else:
                        dsegs = [(0, nd)]
                    segs = [(a, b, nc.vector) for a, b in dsegs] + [
                        (nd, nd + ng1, nc.gpsimd),
                        (nd + ng1, NPP, nc.gpsimd),
                    ]
                    for n0, n1, eng in segs:
                        ns = slice(n0, n1)
                        H = n1 - n0
                        if eng is nc.vector:
                            mid_b = pm_rep[:, 0:D] \
                                .unsqueeze(1).broadcast_to([P, H, D])
                            rinv_b = pm_rep[:, D:2 * D] \
                                .unsqueeze(1).broadcast_to([P, H, D])
                        else:
                            mid_b = rep_sb[:, 0:D] \
                                .unsqueeze(1).broadcast_to([P, H, D])
                            rinv_b = rep_sb[:, D:2 * D] \
                                .unsqueeze(1).broadcast_to([P, H, D])
                        eng.tensor_tensor(
                            tv3[:, ns, :], tv3[:, ns, :], mid_b, op=OP.subtract
                        )
                        eng.tensor_tensor(
                            tv3[:, ns, :], tv3[:, ns, :], rinv_b, op=OP.mult
                        )
                        if eng is nc.vector:
                            nc.scalar.dma_start(
                                yr[:, n0 * D:n1 * D], tp[:, n0 * D:n1 * D]
                            )
                        else:
                            nc.gpsimd.dma_start(
                                yr[:, n0 * D:n1 * D], tp[:, n0 * D:n1 * D]
                            )

                if r < ROUNDS:
                    # cross-partition fold tree + stat math on PSUM scratch
                    # (in0 always a single SBUF stream, in1/out PSUM). Borrows
                    # the fold-acc pool slot (same 9600 B/partition; the fold
                    # chains of this round are done with it by now) - only
                    # pages [0:2] of the [GPR, 4, 2D] view are used.
                    scr = acc_pool.tile([GPR, 2, 2 * D], F32, tag="acc")
                    nc.vector.tensor_copy(scr[:], tq[:, 2:4, :])
                    nc.vector.tensor_tensor(
                        scr[:, :, 0:D], tq[:, 0:2, 0:D], scr[:, :, 0:D],
                        op=OP.max,
                    )
                    nc.vector.tensor_tensor(
                        scr[:, :, D:2 * D], tq[:, 0:2, D:2 * D],
                        scr[:, :, D:2 * D], op=OP.min,
                    )
                    h2 = sml_pool.tile([GPR, 2 * D], F32, tag="h2")
                    nc.vector.tensor_copy(h2[:], scr[:, 1, :])
                    nc.vector.tensor_tensor(
                        scr[:, 0, 0:D], h2[:, 0:D], scr[:, 0, 0:D], op=OP.max
                    )
                    nc.vector.tensor_tensor(
                        scr[:, 0, D:2 * D], h2[:, D:2 * D], scr[:, 0, D:2 * D],
                        op=OP.min,
                    )
                    # pmax = scr[:,0,0:D], pmin = scr[:,0,D:2D] (PSUM)
                    pmin_sb = sml_pool.tile([GPR, D], F32, tag="pminsb")
                    nc.vector.tensor_copy(pmin_sb[:], scr[:, 0, D:2 * D])
                    # mid = (pmax+pmin)*0.5 -> scr[:,1,0:D]
                    nc.vector.tensor_tensor(
                        scr[:, 1, 0:D], pmin_sb[:], scr[:, 0, 0:D], op=OP.add
                    )
                    nc.vector.tensor_tensor(
                        scr[:, 1, 0:D], half_b, scr[:, 1, 0:D], op=OP.mult
                    )
                    # rinv = 1/max((pmin-pmax)*-0.5, EPS) -> scr[:,1,D:2D]
                    nc.vector.tensor_tensor(
                        scr[:, 1, D:2 * D], pmin_sb[:], scr[:, 0, 0:D],
                        op=OP.subtract,
                    )
                    nc.vector.tensor_tensor(
                        scr[:, 1, D:2 * D], neghalf_b, scr[:, 1, D:2 * D],
                        op=OP.mult,
                    )
                    nc.vector.tensor_tensor(
                        scr[:, 1, D:2 * D], eps_b, scr[:, 1, D:2 * D], op=OP.max
                    )
                    nc.vector.reciprocal(scr[:, 0, 0:D], scr[:, 1, D:2 * D])
                    # ab_sb = (mid, rinv) on 32 partitions
                    ab = sml_pool.tile([GPR, 2 * D], F32, tag="ab")
                    nc.vector.tensor_copy(ab[:, 0:D], scr[:, 1, 0:D])
                    nc.vector.tensor_copy(ab[:, D:2 * D], scr[:, 0, 0:D])

                    # replicate stats to all Q partitions of each graph
                    # (scalar ring; only early-completing work lives there)
                    rep_sb = rep_pool.tile([P, 2 * D], F32, tag="repsb")
                    for q in range(Q):
                        nc.scalar.dma_start(rep_sb[q::Q, :], ab[:, :])
                    # and into PSUM for the DVE slice (DVE copy: ACT
                    # would block under GpSimd)
                    nc.vector.tensor_copy(pm_rep[:], rep_sb[:])

                    live[r] = (t, rep_sb)

    _split_multi_waits(nc, mybir)
    return nc


def kernel(tensor, batch_list=None, **_ignored):
    """Full-input entry point: tensor [262144, 300] fp32 -> [262144, 300] fp32.

    batch_list is the constant 256-per-graph layout baked into this kernel.
    """
    from concourse.bass_utils import run_bass_kernel_spmd

    tensor = np.ascontiguousarray(np.asarray(tensor), dtype=np.float32)
    assert tensor.shape == (NUM_GRAPHS * NPG, D), tensor.shape

    if "nc" not in _CACHE:
        _CACHE["nc"] = _build()
    nc = _CACHE["nc"]

    in_maps = [
        {"x": tensor[c * ROWS_PER_CORE:(c + 1) * ROWS_PER_CORE]}
        for c in range(N_CORES)
    ]
    res = run_bass_kernel_spmd(nc, in_maps, core_ids=list(range(N_CORES)))
    out = np.concatenate([res.results[c]["y"] for c in range(N_CORES)], axis=0)
    return out


# revision 22
# speedup vs baseline: 1.0224x; 1.0224x over previous
"""Trainium2 Bass kernel for nn_Cube_Norm (segment min/max normalize).

Reference semantics (per graph g of 256 nodes, per dim d):
    tmax = max_n x[g,n,d]; tmin = min_n x[g,n,d]
    mid = (tmax+tmin)/2; ldv = max((tmax-tmin)/2, 1e-12)
    out[g,n,d] = (x[g,n,d] - mid) / ldv

Sharding: 1024 graphs -> 8 cores x 128 graphs (row-sharded at graph
boundaries). Per core, 4 rounds of 32 graphs; each graph occupies 4 SBUF
partitions (64 nodes each): every round is a [128, 19200] fp32 tile with
contiguous DMA in/out (exactly 2x HBM traffic), double-buffered.

Engine split (HW-probed on this silicon):
  - A DVE op with two SBUF operands (or an sb->sb copy) crawls 4-10x
    while GpSimd is busy: they arbitrate a shared SBUF port pair and the
    loser blocks for the whole instruction. A DVE op with one SBUF
    stream + PSUM for the rest runs at FULL speed alongside GpSimd.
  - So: DVE folds are PSUM-accumulator chains TT(chunk_sb, acc_pm ->
    acc_pm); the cross-partition stat tree + math run on PSUM scratch
    (with small ACT sb<->psum copies; ACT has its own ports); the
    DVE normalize slice reads stats from PSUM. GpSimd runs the bulk of
    the normalize from SBUF concurrently; in round 0 (no normalize yet)
    it folds a share of the chunks instead.
  - All stat math is TT/reciprocal with broadcast const tiles - never
    tensor_scalar/copy on DVE (2-port modes would grab the shared pair).
  - Loads ride the sync HWDGE ring; stores and stat DMAs ride the
    scalar ring, so stores never head-of-line-block loads.
"""

import numpy as np

NUM_GRAPHS = 1024
NPG = 256            # nodes per graph
D = 300              # embed dim
N_CORES = 8
GPC = NUM_GRAPHS // N_CORES   # 128 graphs per core
ROWS_PER_CORE = GPC * NPG     # 32768
P = 128              # SBUF partitions
Q = 4                # partitions per graph
NPP = NPG // Q       # 64 nodes per partition
GPR = P // Q         # 32 graphs per round
ROUNDS = GPC // GPR  # 4
FREE = NPP * D       # 19200 fp32 per partition per round
ROWS_PER_ROUND = GPR * NPG    # 8192
EPS = 1e-12

CH = 1200            # fold chunk (PSUM acc width; 300*2^k)
NCH = FREE // CH     # 16 chunks

# normalize node-split: DVE handles ND_* nodes of 64, GpSimd the rest
ND_MID = 12
ND_LAST = 43

_CACHE = {}


def _split_multi_waits(nc, mybir, max_waits=1):
    """walrus in this container rejects >N sync waits on one instruction;
    hoist extras into standalone NOPs on the same engine just before."""
    n = 0
    for f in nc.m.functions:
        for bb in f.blocks:
            new_insts = []
            for inst in bb.instructions:
                si = inst.sync_info
                if si is not None and si.on_wait and len(si.on_wait) > max_waits:
                    extra = list(si.on_wait[: len(si.on_wait) - max_waits])
                    keep = list(si.on_wait[len(si.on_wait) - max_waits:])
                    for j, w in enumerate(extra):
                        new_insts.append(
                            mybir.InstNoOp(
                                name=f"{inst.name}-sw{j}",
                                sync_info=mybir.SyncInfo(on_wait=[w], on_update=[]),
                                bass_nofuse=True,
                                engine=inst.engine,
                            )
                        )
                        n += 1
                    inst.sync_info = mybir.SyncInfo(
                        on_wait=keep, on_update=list(si.on_update)
                    )
                new_insts.append(inst)
            bb.instructions.clear()
            for i in new_insts:
                bb.add_instruction(i)
    return n


def _build():
    import concourse.bass as bass
    import concourse.tile as tile
    from concourse import mybir

    F32 = mybir.dt.float32
    OP = mybir.AluOpType

    nc = bass.Bass()
    x = nc.dram_tensor("x", [ROWS_PER_CORE, D], F32, kind="ExternalInput")
    y = nc.dram_tensor("y", [ROWS_PER_CORE, D], F32, kind="ExternalOutput")

    with tile.TileContext(nc) as tc:
        with tc.tile_pool(name="data", bufs=2) as data_pool, \
             tc.tile_pool(name="rep", bufs=2) as rep_pool, \
             tc.tile_pool(name="sml", bufs=1) as sml_pool, \
             tc.tile_pool(name="acc", bufs=2, space="PSUM") as acc_pool, \
             tc.tile_pool(name="prep", bufs=1, space="PSUM") as prep_pool:
            # broadcast consts for stat math ([GPR,1], read via rd0)
            cst = sml_pool.tile([GPR, 4], F32, tag="cst")
            nc.vector.memset(cst[:, 0:1], 0.5)
            nc.vector.memset(cst[:, 1:2], -0.5)
            nc.vector.memset(cst[:, 2:3], EPS)
            half_b = cst[:, 0:1].broadcast_to([GPR, D])
            neghalf_b = cst[:, 1:2].broadcast_to([GPR, D])
            eps_b = cst[:, 2:3].broadcast_to([GPR, D])

            # persistent PSUM stats [P, mid|rinv] (single buffer: its
            # reader (norm r-1) and writer (stats r) are both in-order
            # DVE ops, so WAR needs no extra buffering)
            pm_rep = prep_pool.tile([P, 2 * D], F32, tag="pmrep")

            live = {}  # r -> (t, rep_sb, parity) awaiting normalize+store
            for r in range(ROUNDS + 1):
                if r < ROUNDS:
                    rows = slice(r * ROWS_PER_ROUND, (r + 1) * ROWS_PER_ROUND)

                    # load in four quarters so folds start as data streams in
                    t = data_pool.tile([P, FREE], F32, tag="t")
                    xr = x[rows, :].rearrange("(p f) d -> p (f d)", p=P)
                    FQ = FREE // 4
                    for qd in range(4):
                        nc.sync.dma_start(
                            t[:, qd * FQ:(qd + 1) * FQ], xr[:, qd * FQ:(qd + 1) * FQ]
                        )

                    # per-partition partials: s cols [0:D]=max, [D:2D]=min.
                    # DVE chain keeps the accumulator in PSUM (in0 = sbuf
                    # chunk via the dedicated read port, in1/out = PSUM), so
                    # it never touches the DVE/GpSimd shared SBUF ports.
                    # ACT fully blocks whenever GpSimd is busy (and GpSimd
                    # runs back-to-back rounds), so ACT is not used at all in
                    # steady state: the acc inits are DVE copies too (sbuf
                    # read + PSUM write never touches the shared pair).
                    # (GpSimd can't help fold: Pool TT rejects max/min ops.)
                    s = sml_pool.tile([P, 2 * D], F32, tag="s")
                    accs = {}
                    for si in (0, 1):
                        accs[si] = acc_pool.tile(
                            [P, CH], F32, tag="acc", name=f"acc{r}_{si}"
                        )
                        nc.vector.tensor_copy(accs[si][:], t[:, 0:CH])
                    for si, op in ((0, OP.max), (1, OP.min)):
                        acc = accs[si]
                        for c in range(1, NCH):
                            nc.vector.tensor_tensor(
                                acc[:], t[:, c * CH:(c + 1) * CH], acc[:], op=op
                            )
                        h = sml_pool.tile([P, CH // 2], F32, tag="h")
                        m = CH // 2
                        while m > D:
                            nc.vector.tensor_copy(h[:, 0:m], acc[:, m:2 * m])
                            nc.vector.tensor_tensor(
                                acc[:, 0:m], h[:, 0:m], acc[:, 0:m], op=op
                            )
                            m //= 2
                        nc.vector.tensor_copy(h[:, 0:D], acc[:, D:2 * D])
                        nc.vector.tensor_tensor(
                            s[:, si * D:(si + 1) * D], h[:, 0:D],
                            acc[:, 0:D], op=op,
                        )

                    # gather the 4 partials of each graph onto one partition.
                    # Scalar ring: the sync ring is reserved for loads so the
                    # next round's loads are never head-of-line-blocked; the
                    # scalar ring carries only early-completing work (acc
                    # inits, the DVE-slice store, gather, replicate).
                    tq = sml_pool.tile([GPR, Q, 2 * D], F32, tag="tq")
                    for q in range(Q):
                        nc.scalar.dma_start(tq[:, q, :], s[q::Q, :])

                if r >= 1:
                    # normalize round r-1: out = (x - mid) * rinv, in place.
                    # DVE slices read stats from PSUM (shared-pair-free) and
                    # store via the scalar ring (they complete early, so they
                    # can't block gather/replicate behind them). GpSimd slices
                    # read stats from SBUF and store via their own SWDGE ring
                    # right after their compute - keeping late-completing
                    # stores off both HWDGE rings entirely.
                    tp, rep_sb = live.pop(r - 1)
                    rowsp = slice((r - 1) * ROWS_PER_ROUND, r * ROWS_PER_ROUND)
                    tv3 = tp[:].rearrange("p (n d) -> p n d", n=NPP, d=D)
                    yr = y[rowsp, :].rearrange("(p f) d -> p (f d)", p=P)

                    nd = ND_LAST if r == ROUNDS else ND_MID
                    ng1 = (NPP - nd + 1) // 2
                    if r == ROUNDS:
                        # drain round: chunk the DVE slice so stores overlap
                        # the remaining compute instead of a serial tail
                        dsegs = [(a, min(a + 15, nd)) for a in range(0, nd, 15)]
                    else:
                        dsegs = [(0, nd)]
                    segs = [(a, b, nc.vector) for a, b in dsegs] + [
                        (nd, nd + ng1, nc.gpsimd),
                        (nd + ng1, NPP, nc.gpsimd),
                    ]
                    for n0, n1, eng in segs:
                        ns = slice(n0, n1)
                        H = n1 - n0
                        if eng is nc.vector:
                            mid_b = pm_rep[:, 0:D] \
                                .unsqueeze(1).broadcast_to([P, H, D])
                            rinv_b = pm_rep[:, D:2 * D] \
                                .unsqueeze(1).broadcast_to([P, H, D])
                        else:
                            mid_b = rep_sb[:, 0:D] \
                                .unsqueeze(1).broadcast_to([P, H, D])
                            rinv_b = rep_sb[:, D:2 * D] \
                                .unsqueeze(1).broadcast_to([P, H, D])
                        eng.tensor_tensor(
                            tv3[:, ns, :], tv3[:, ns, :], mid_b, op=OP.subtract
                        )
                        eng.tensor_tensor(
                            tv3[:, ns, :], tv3[:, ns, :], rinv_b, op=OP.mult
                        )
                        if eng is nc.vector:
                            nc.scalar.dma_start(
                                yr[:, n0 * D:n1 * D], tp[:, n0 * D:n1 * D]
                            )
                        else:
                            nc.gpsimd.dma_start(
                                yr[:, n0 * D:n1 * D], tp[:, n0 * D:n1 * D]
                            )

                if r < ROUNDS:
                    # cross-partition fold tree + stat math on PSUM scratch
                    # (in0 always a single SBUF stream, in1/out PSUM). Borrows
                    # the fold-acc pool slot (same 9600 B/partition; the fold
                    # chains of this round are done with it by now) - only
                    # pages [0:2] of the [GPR, 4, 2D] view are used.
                    scr = acc_pool.tile([GPR, 2, 2 * D], F32, tag="acc")
                    nc.vector.tensor_copy(scr[:], tq[:, 2:4, :])
                    nc.vector.tensor_tensor(
                        scr[:, :, 0:D], tq[:, 0:2, 0:D], scr[:, :, 0:D],
                        op=OP.max,
                    )
                    nc.vector.tensor_tensor(
                        scr[:, :, D:2 * D], tq[:, 0:2, D:2 * D],
                        scr[:, :, D:2 * D], op=OP.min,
                    )
                    h2 = sml_pool.tile([GPR, 2 * D], F32, tag="h2")
                    nc.vector.tensor_copy(h2[:], scr[:, 1, :])
                    nc.vector.tensor_tensor(
                        scr[:, 0, 0:D], h2[:, 0:D], scr[:, 0, 0:D], op=OP.max
                    )
                    nc.vector.tensor_tensor(
                        scr[:, 0, D:2 * D], h2[:, D:2 * D], scr[:, 0, D:2 * D],
                        op=OP.min,
                    )
                    # pmax = scr[:,0,0:D], pmin = scr[:,0,D:2D] (PSUM)
                    pmin_sb = sml_pool.tile([GPR, D], F32, tag="pminsb")
                    nc.vector.tensor_copy(pmin_sb[:], scr[:, 0, D:2 * D])
                    # mid = (pmax+pmin)*0.5 -> scr[:,1,0:D]
                    nc.vector.tensor_tensor(
                        scr[:, 1, 0:D], pmin_sb[:], scr[:, 0, 0:D], op=OP.add
                    )
                    nc.vector.tensor_tensor(
                        scr[:, 1, 0:D], half_b, scr[:, 1, 0:D], op=OP.mult
                    )
                    # rinv = 1/max((pmin-pmax)*-0.5, EPS) -> scr[:,1,D:2D]
                    nc.vector.tensor_tensor(
                        scr[:, 1, D:2 * D], pmin_sb[:], scr[:, 0, 0:D],
                        op=OP.subtract,
                    )
                    nc.vector.tensor_tensor(
                        scr[:, 1, D:2 * D], neghalf_b, scr[:, 1, D:2 * D],
                        op=OP.mult,
                    )
                    nc.vector.tensor_tensor(
                        scr[:, 1, D:2 * D], eps_b, scr[:, 1, D:2 * D], op=OP.max
                    )
                    nc.vector.reciprocal(scr[:, 0, 0:D], scr[:, 1, D:2 * D])
                    # ab_sb = (mid, rinv) on 32 partitions
                    ab = sml_pool.tile([GPR, 2 * D], F32, tag="ab")
                    nc.vector.tensor_copy(ab[:, 0:D], scr[:, 1, 0:D])
                    nc.vector.tensor_copy(ab[:, D:2 * D], scr[:, 0, 0:D])

                    # replicate stats to all Q partitions of each graph
                    # (scalar ring; only early-completing work lives there)
                    rep_sb = rep_pool.tile([P, 2 * D], F32, tag="repsb")
                    for q in range(Q):
                        nc.scalar.dma_start(rep_sb[q::Q, :], ab[:, :])
                    # and into PSUM for the DVE slice (DVE copy: ACT
                    # would block under GpSimd)
                    nc.vector.tensor_copy(pm_rep[:], rep_sb[:])

                    live[r] = (t, rep_sb)

    _split_multi_waits(nc, mybir)
    return nc


def kernel(tensor, batch_list=None, **_ignored):
    """Full-input entry point: tensor [262144, 300] fp32 -> [262144, 300] fp32.

    batch_list is the constant 256-per-graph layout baked into this kernel.
    """
    from concourse.bass_utils import run_bass_kernel_spmd

    tensor = np.ascontiguousarray(np.asarray(tensor), dtype=np.float32)
    assert tensor.shape == (NUM_GRAPHS * NPG, D), tensor.shape

    if "nc" not in _CACHE:
        _CACHE["nc"] = _build()
    nc = _CACHE["nc"]

    in_maps = [
        {"x": tensor[c * ROWS_PER_CORE:(c + 1) * ROWS_PER_CORE]}
        for c in range(N_CORES)
    ]
    res = run_bass_kernel_spmd(nc, in_maps, core_ids=list(range(N_CORES)))
    out = np.concatenate([res.results[c]["y"] for c in range(N_CORES)], axis=0)
    return out
